# revision 1
# baseline (speedup 1.0000x reference)
"""
Trainium2 Bass kernel for the Decoder_RNN_Simple problem.

Math (per flat-batch element b, reference semantics):
  hidden0 = tanh(W_z0 @ z0 + b_z0)                       # [256]
  cur0 = 0
  for t in 0..199:
    x = [cur, tps[t]]                                    # [65]
    gx = W_ih @ x + b_ih ; gh = W_hh @ hidden + b_hh     # [768]
    r = sig(gx_r + gh_r); z = sig(gx_z + gh_z)
    n = tanh(gx_n + r * gh_n)
    h' = (1-z)*n + z*h ; pred = W_out @ h' + b_out       # [64]

Mapping (data-parallel over the flat batch of 8192 across 8 cores,
1024 rows per core; transposed [gates, batch] on-chip layout so the
recurrence needs no transposes):

  - cur_t = pred_{t-1} = W_out @ h_t + b_out for t>=1, so the r/z gate
    pre-activations fold into a single matmul with
    W_eff = W_hh + W_ih[:, :64] @ W_out applied to h (exact algebra).
    The n gate keeps xn (from the pred tile, K=64) and hn (from raw
    W_hh) separate since n = tanh(xn + r*hn).
  - All t-dependent bias terms (b_ih + b_hh + W_ih[:,64]*tps[t] (+
    W_ih[:, :64] @ b_out for t>=1)) are precomputed host-side as
    [gate, 200] tables and applied through the ACT bias operand.
"""

import sys

_TRN = "/opt/trn_rl_repo"
if _TRN not in sys.path:
    sys.path.insert(0, _TRN)

import numpy as np

import concourse.bass as bass
import concourse.mybir as mybir
import concourse.tile as tile
from concourse.vector_clock import ScopedClock
from concourse.bass_utils import run_bass_kernel_spmd

N_CORES = 8
LATENT = 128
OUT_DIM = 64
N_GRU = 256
N_TP = 200
B_FULL = 64 * 128
B_LOC = B_FULL // N_CORES  # 1024
HALF = 512
F32 = mybir.dt.float32
F32R = mybir.dt.float32r


def _mm(ap):
    return ap  # tiles feeding the PE are allocated as float32r directly
AF = mybir.ActivationFunctionType
ALU = mybir.AluOpType


# walrus rejects sem waits carried on the kernel-tail Drain instruction
# ("Too many sync wait commands"); move them onto NOPs, one wait each.
def _patched_drain_and_barrier(self, tick_clock, wait_clock):
    carrier = self.nc.sync.nop()
    wait_clock.add_sem_waits(carrier.ins, ScopedClock({None: tick_clock.global_clock}))
    si = carrier.ins.sync_info
    waits = list(si.on_wait) if si is not None else []
    if len(waits) > 1:
        si.on_wait = waits[:1]
        rest = waits[1:]
        while rest:
            extra = self.nc.sync.nop()
            extra.ins.sync_info = mybir.SyncInfo(on_wait=rest[:1], on_update=[])
            rest = rest[1:]
    self.nc.sync.drain()
    self.nc.all_engine_barrier()
    popped = self.nc._tile_sem_poison_stack.pop()
    assert popped is self._sem_poison
    self.nc.clear_and_free_semaphores(list(self.sems.allocated().values()))
    self.nc.all_engine_barrier()


tile.TileContext._drain_and_barrier = _patched_drain_and_barrier


def _split_waits(nc, maxw=1):
    """This walrus rejects instructions carrying more than a couple of sem
    waits; move the excess onto same-engine NOPs inserted just before."""
    k = 0
    for f in nc.m.functions:
        for bb in f.blocks:
            insts = bb.instructions
            out = []
            changed = False
            for inst in insts:
                si = inst.sync_info
                waits = list(si.on_wait) if si is not None else []
                if len(waits) > maxw:
                    si.on_wait = waits[-maxw:]
                    excess = waits[:-maxw]
                    while excess:
                        chunk, excess = excess[:maxw], excess[maxw:]
                        nop = mybir.InstNoOp(name=f"waitsplit_{k}", ins=[], outs=[])
                        k += 1
                        nop.engine = inst.engine
                        nop.sync_info = mybir.SyncInfo(on_wait=chunk, on_update=[])
                        out.append(nop)
                    changed = True
                out.append(inst)
            if changed:
                bb.instructions = out
    return k


def _build_module(repeat=1):
    nc = bass.Bass("TRN2", target_bir_lowering=False, debug=False, num_devices=N_CORES)

    def inp(name, shape):
        return nc.dram_tensor(name, shape, F32, kind="ExternalInput").ap()

    d = {
        "z0t": inp("z0t", [LATENT, B_LOC]),
        "wz0t": inp("wz0t", [LATENT, N_GRU]),
        "whht1": inp("whht1", [N_GRU, 3 * N_GRU]),  # eff for r,z; raw for n
        "whht0": inp("whht0", [N_GRU, 2 * N_GRU]),  # raw r,z (step 0)
        "wxnt": inp("wxnt", [OUT_DIM, N_GRU]),
        "woutt": inp("woutt", [N_GRU, OUT_DIM]),
        "brz": inp("brz", [2 * N_GRU, N_TP]),
        "bxn": inp("bxn", [N_GRU, N_TP]),
        "bhhn": inp("bhhn", [N_GRU, 1]),
        "bz0": inp("bz0", [N_GRU, 1]),
        "bout": inp("bout", [OUT_DIM, 1]),
    }
    out = nc.dram_tensor("out", [N_TP, OUT_DIM, B_LOC], F32, kind="ExternalOutput").ap()

    with tile.TileContext(nc) as tc:
        for _ in range(repeat):
            _emit(nc, tc, d, out)
    n = _split_waits(nc, maxw=1)
    print(f"[kernel] split {n} excess sem-waits onto NOPs", flush=True)
    return nc


NSTREAM = 2  # independent sub-batch recurrences per core (hides serial chain)


def _emit(nc, tc, d, out):
    SW = B_LOC // NSTREAM  # batch columns per stream
    ns = NSTREAM
    with (
        tc.tile_pool(name="const", bufs=1) as cp,
        tc.tile_pool(name="work", bufs=2) as wp,
        tc.tile_pool(name="psum", bufs=6, space="PSUM") as pp,
        tc.tile_pool(name="ppred", bufs=2, space="PSUM") as ppr,
    ):
        def const_tile(name, shape, dt=F32):
            t = cp.tile(shape, dt, tag=name)
            dma = nc.gpsimd if dt is F32R else nc.sync
            dma.dma_start(t[:], d[name][:])
            return t

        def const_rows(name, shape, r0, tag, dt=F32):
            t = cp.tile(shape, dt, tag=tag)
            dma = nc.gpsimd if dt is F32R else nc.sync
            dma.dma_start(t[:], d[name][r0 : r0 + shape[0], :])
            return t

        wz0 = const_tile("wz0t", [LATENT, N_GRU], F32R)
        whh1 = [const_rows("whht1", [128, 3 * N_GRU], 128 * k, f"whh1_{k}", F32R) for k in range(2)]
        whh0 = [const_rows("whht0", [128, 2 * N_GRU], 128 * k, f"whh0_{k}", F32R) for k in range(2)]
        wxn = const_tile("wxnt", [OUT_DIM, N_GRU], F32R)
        wout = [const_rows("woutt", [128, OUT_DIM], 128 * k, f"wout_{k}", F32R) for k in range(2)]
        brz = [const_rows("brz", [128, N_TP], 128 * g, f"brz_{g}") for g in range(4)]
        bxn = [const_rows("bxn", [128, N_TP], 128 * c, f"bxn_{c}") for c in range(2)]
        bhhn = [const_rows("bhhn", [128, 1], 128 * c, f"bhhn_{c}") for c in range(2)]
        bz0 = [const_rows("bz0", [128, 1], 128 * c, f"bz0_{c}") for c in range(2)]
        bout = const_tile("bout", [OUT_DIM, 1])

        # ---- initial hidden: h = tanh(Wz0 @ z0T + b_z0), [256, B] as 2 chunks
        z0sb = wp.tile([LATENT, B_LOC], F32R, tag="z0")
        nc.gpsimd.dma_start(z0sb[:], d["z0t"][:])
        # h[s][c]: stream s, gate chunk c -> [128, SW]
        h = [[None, None] for _ in range(ns)]
        for s in range(ns):
            bs = slice(s * SW, (s + 1) * SW)
            for c in range(2):
                p = pp.tile([128, SW], F32, tag="ps")
                nc.tensor.matmul(p[:], _mm(wz0[:, c * 128 : (c + 1) * 128]), _mm(z0sb[:, bs]),
                                 start=True, stop=True)
                hc = wp.tile([128, SW], F32R, tag=f"h{c}_{s}")
                nc.scalar.activation(hc[:], p[:], AF.Tanh, bias=bz0[c][:, 0:1])
                h[s][c] = hc

        pred = [None] * ns
        for t in range(N_TP):
            first = t == 0
            wk = whh0 if first else whh1

            # r and z gate chunks: accumulate (W_eff @ h) in PSUM, sigmoid out
            sig = [[None] * 4 for _ in range(ns)]
            for g in range(4):  # r0 r1 z0 z1
                col = slice(g * 128, (g + 1) * 128)
                for s in range(ns):
                    p = pp.tile([128, SW], F32, tag="ps")
                    nc.tensor.matmul(p[:], _mm(wk[0][:, col]), _mm(h[s][0][:]),
                                     start=True, stop=False)
                    nc.tensor.matmul(p[:], _mm(wk[1][:, col]), _mm(h[s][1][:]),
                                     start=False, stop=True)
                    sg = wp.tile([128, SW], F32, tag=f"sig{g}_{s}")
                    nc.scalar.activation(sg[:], p[:], AF.Sigmoid,
                                         bias=brz[g][:, t : t + 1])
                    sig[s][g] = sg

            # n gate: hn (raw W_hh) and xn (from pred, K=64) kept separate
            phn = [[None, None] for _ in range(ns)]
            pxn = [[None, None] for _ in range(ns)]
            for c in range(2):
                col = slice(512 + c * 128, 512 + (c + 1) * 128)
                xcol = slice(c * 128, (c + 1) * 128)
                for s in range(ns):
                    p = pp.tile([128, SW], F32, tag="ps")
                    nc.tensor.matmul(p[:], _mm(whh1[0][:, col]), _mm(h[s][0][:]),
                                     start=True, stop=False)
                    nc.tensor.matmul(p[:], _mm(whh1[1][:, col]), _mm(h[s][1][:]),
                                     start=False, stop=True)
                    phn[s][c] = p
                    if not first:
                        px = pp.tile([128, SW], F32, tag="ps")
                        nc.tensor.matmul(px[:], _mm(wxn[:, xcol]), _mm(pred[s][:]),
                                         start=True, stop=True)
                        pxn[s][c] = px

            h_new = [[None, None] for _ in range(ns)]
            for c in range(2):
                for s in range(ns):
                    # t1 = (hn + b_hhn) * r   (one fused DVE op)
                    t1 = wp.tile([128, SW], F32, tag=f"t1_{c}_{s}")
                    nc.vector.scalar_tensor_tensor(
                        t1[:], phn[s][c][:], bhhn[c][:, 0:1], sig[s][c][:],
                        ALU.add, ALU.mult,
                    )
                    if first:
                        t2 = t1
                    else:
                        t2 = wp.tile([128, SW], F32, tag=f"t2_{c}_{s}")
                        nc.vector.tensor_tensor(t2[:], t1[:], pxn[s][c][:], ALU.add)
                    n = wp.tile([128, SW], F32, tag=f"n_{c}_{s}")
                    nc.scalar.activation(n[:], t2[:], AF.Tanh,
                                         bias=bxn[c][:, t : t + 1])
                    # h' = n + z*(h-n); the sub runs on GPSIMD to unload DVE
                    dt_ = wp.tile([128, SW], F32, tag=f"d_{c}_{s}")
                    nc.gpsimd.tensor_sub(dt_[:], h[s][c][:], n[:])
                    e = wp.tile([128, SW], F32, tag=f"e_{c}_{s}")
                    nc.vector.tensor_mul(e[:], sig[s][2 + c][:], dt_[:])
                    hc = wp.tile([128, SW], F32R, tag=f"h{c}_{s}")
                    nc.vector.tensor_add(hc[:], e[:], n[:])
                    h_new[s][c] = hc
            h = h_new

            # pred = W_out @ h' + b_out  -> DRAM out[t], and rhs for next xn
            for s in range(ns):
                bs = slice(s * SW, (s + 1) * SW)
                p = ppr.tile([OUT_DIM, SW], F32, tag="pp")
                nc.tensor.matmul(p[:], _mm(wout[0][:, :]), _mm(h[s][0][:]),
                                 start=True, stop=False)
                nc.tensor.matmul(p[:], _mm(wout[1][:, :]), _mm(h[s][1][:]),
                                 start=False, stop=True)
                pr = wp.tile([OUT_DIM, SW], F32R, tag=f"pred_{s}")
                nc.scalar.activation(pr[:], p[:], AF.Identity, bias=bout[:, 0:1])
                pred[s] = pr
                nc.sync.dma_start(out[t][:, bs], pr[:].bitcast(F32))


_CACHE = {}


def _prep_host(z0, tps_to_pred, W_z0, b_z0, W_ih, b_ih, W_hh, b_hh, W_out, b_out):
    f = np.float32
    z0 = np.asarray(z0, f)
    tps = np.asarray(tps_to_pred, f)
    W_z0, b_z0 = np.asarray(W_z0, f), np.asarray(b_z0, f)
    W_ih, b_ih = np.asarray(W_ih, f), np.asarray(b_ih, f)
    W_hh, b_hh = np.asarray(W_hh, f), np.asarray(b_hh, f)
    W_out, b_out = np.asarray(W_out, f), np.asarray(b_out, f)

    Wihp = W_ih[:, :OUT_DIM]  # [768, 64]
    wt = W_ih[:, OUT_DIM]  # [768]
    G2 = 2 * N_GRU
    Weff_rz = W_hh[:G2] + Wihp[:G2] @ W_out  # [512, 256]
    whht1 = np.ascontiguousarray(
        np.concatenate([Weff_rz, W_hh[G2:]], axis=0).T
    )  # [256, 768]
    whht0 = np.ascontiguousarray(W_hh[:G2].T)  # [256, 512]
    wxnt = np.ascontiguousarray(Wihp[G2:].T)  # [64, 256]
    woutt = np.ascontiguousarray(W_out.T)  # [256, 64]

    cb = Wihp @ b_out  # [768]
    bias_all = b_ih[:, None] + wt[:, None] * tps[None, :]  # [768, 200]
    brz = bias_all[:G2] + b_hh[:G2, None]
    brz[:, 1:] += cb[:G2, None]
    bxn = bias_all[G2:].copy()
    bxn[:, 1:] += cb[G2:, None]

    shared = {
        "wz0t": np.ascontiguousarray(W_z0.T),
        "whht1": whht1,
        "whht0": whht0,
        "wxnt": wxnt,
        "woutt": woutt,
        "brz": np.ascontiguousarray(brz, f),
        "bxn": np.ascontiguousarray(bxn, f),
        "bhhn": np.ascontiguousarray(b_hh[G2:].reshape(N_GRU, 1)),
        "bz0": np.ascontiguousarray(b_z0.reshape(N_GRU, 1)),
        "bout": np.ascontiguousarray(b_out.reshape(OUT_DIM, 1)),
    }
    z0f = z0.reshape(B_FULL, LATENT)
    in_maps = []
    for i in range(N_CORES):
        m = dict(shared)
        m["z0t"] = np.ascontiguousarray(z0f[i * B_LOC : (i + 1) * B_LOC].T)
        in_maps.append(m)
    return in_maps


def _run(in_maps, repeat=1, **spmd_kwargs):
    key = f"nc{repeat}"
    if key not in _CACHE:
        _CACHE[key] = _build_module(repeat)
    return run_bass_kernel_spmd(_CACHE[key], in_maps, list(range(N_CORES)), **spmd_kwargs)


def _gather(res):
    outp = np.empty((B_FULL, N_TP, OUT_DIM), np.float32)
    for i in range(N_CORES):
        o = res.results[i]["out"]  # [200, 64, 1024]
        outp[i * B_LOC : (i + 1) * B_LOC] = np.asarray(o).transpose(2, 0, 1)
    return outp.reshape(64, 128, N_TP, OUT_DIM)


def kernel(**inputs):
    in_maps = _prep_host(**inputs)
    res = _run(in_maps)
    return _gather(res)


def kernel_profiled(**inputs):
    """Like kernel(), but requests an NTFF trace; returns (output, results)."""
    in_maps = _prep_host(**inputs)
    res = _run(in_maps, trace=True)
    return _gather(res), res



# revision 5
# speedup vs baseline: 1.1887x; 1.1887x over previous
"""
Trainium2 Bass kernel for the Decoder_RNN_Simple problem (v2).

Math (per flat-batch element b, reference semantics):
  hidden0 = tanh(W_z0 @ z0 + b_z0)                       # [256]
  cur0 = 0
  for t in 0..199:
    x = [cur, tps[t]]                                    # [65]
    gx = W_ih @ x + b_ih ; gh = W_hh @ hidden + b_hh     # [768]
    r = sig(gx_r + gh_r); z = sig(gx_z + gh_z)
    n = tanh(gx_n + r * gh_n)
    h' = (1-z)*n + z*h ; pred = W_out @ h' + b_out       # [64]

Mapping (data-parallel over the flat batch of 8192 across 8 cores,
1024 rows per core, 2 independent 512-column streams per core;
[gates, batch] on-chip layout, bf16 everywhere except PSUM/biases):

  - cur_t = pred_{t-1} = W_out @ h_t + b_out for t>=1 is folded into
    ALL gates: r/z use W_eff = W_hh_rz + W_ih_rz@W_out; the n-gate x
    part uses W_xnh = W_ih_n@W_out so pred never feeds back on-chip.
    The n gate needs xn separate from hn: n = tanh(xn + r*(hn+b_hhn)).
  - t-dependent bias terms are host-precomputed [gate, 200] tables and
    applied through the ACT bias operand.
  - The n-gate sum (xn + t1) is formed in PSUM: the DVE STT writes
    t1 = (hn+b_hhn)*r into a PSUM bank whose has_written bits are
    pre-primed, then the xn matmuls accumulate onto it (start=False).
  - pred = W_out@h' + (b_out added on host) is DMA'd straight from
    PSUM to DRAM; it is not on the recurrence critical path.
"""

import sys

_TRN = "/opt/trn_rl_repo"
if _TRN not in sys.path:
    sys.path.insert(0, _TRN)

import numpy as np

import concourse.bass as bass
import concourse.mybir as mybir
import concourse.tile as tile
from concourse.vector_clock import ScopedClock
from concourse.bass_utils import run_bass_kernel_spmd

N_CORES = 8
LATENT = 128
OUT_DIM = 64
N_GRU = 256
N_TP = 200
B_FULL = 64 * 128
B_LOC = B_FULL // N_CORES  # 1024
SW = 512  # batch columns per stream
NS = 2  # streams
F32 = mybir.dt.float32
BF16 = mybir.dt.bfloat16
AF = mybir.ActivationFunctionType
ALU = mybir.AluOpType

# If the PE-accumulate-onto-DVE-written-PSUM trick fails on HW, set True
# to fall back to an explicit DVE add for t2 = t1 + xn.
USE_TT_ADD = False
# Engine for the blend's d = h - n: "gpsimd" or "dve"
SUB_ENGINE = "gpsimd"


# walrus rejects sem waits carried on the kernel-tail Drain instruction
# ("Too many sync wait commands"); move them onto NOPs, one wait each.
def _patched_drain_and_barrier(self, tick_clock, wait_clock):
    carrier = self.nc.sync.nop()
    wait_clock.add_sem_waits(carrier.ins, ScopedClock({None: tick_clock.global_clock}))
    si = carrier.ins.sync_info
    waits = list(si.on_wait) if si is not None else []
    if len(waits) > 1:
        si.on_wait = waits[:1]
        rest = waits[1:]
        while rest:
            extra = self.nc.sync.nop()
            extra.ins.sync_info = mybir.SyncInfo(on_wait=rest[:1], on_update=[])
            rest = rest[1:]
    self.nc.sync.drain()
    self.nc.all_engine_barrier()
    popped = self.nc._tile_sem_poison_stack.pop()
    assert popped is self._sem_poison
    self.nc.clear_and_free_semaphores(list(self.sems.allocated().values()))
    self.nc.all_engine_barrier()


tile.TileContext._drain_and_barrier = _patched_drain_and_barrier


def _split_waits(nc, maxw=1):
    """This walrus rejects instructions carrying more than a couple of sem
    waits; move the excess onto same-engine NOPs inserted just before."""
    k = 0
    for f in nc.m.functions:
        for bb in f.blocks:
            insts = bb.instructions
            out = []
            changed = False
            for inst in insts:
                si = inst.sync_info
                waits = list(si.on_wait) if si is not None else []
                if len(waits) > maxw:
                    si.on_wait = waits[-maxw:]
                    excess = waits[:-maxw]
                    while excess:
                        chunk, excess = excess[:maxw], excess[maxw:]
                        nop = mybir.InstNoOp(name=f"waitsplit_{k}", ins=[], outs=[])
                        k += 1
                        nop.engine = inst.engine
                        nop.sync_info = mybir.SyncInfo(on_wait=chunk, on_update=[])
                        out.append(nop)
                    changed = True
                out.append(inst)
            if changed:
                bb.instructions = out
    return k


def _build_module():
    nc = bass.Bass("TRN2", target_bir_lowering=False, debug=False, num_devices=N_CORES)

    def inp(name, shape, dt=F32):
        return nc.dram_tensor(name, shape, dt, kind="ExternalInput").ap()

    d = {
        # bf16 operands
        "z0t": inp("z0t", [LATENT, B_LOC], BF16),
        "wz0t": inp("wz0t", [LATENT, N_GRU], BF16),
        # [256, 1024]: cols 0:256 r(eff) 256:512 z(eff) 512:768 hn 768:1024 xnh
        "w1t": inp("w1t", [N_GRU, 4 * N_GRU], BF16),
        # [256, 512]: raw W_hh r,z for step 0
        "w0t": inp("w0t", [N_GRU, 2 * N_GRU], BF16),
        "woutt": inp("woutt", [N_GRU, OUT_DIM], BF16),
        # f32 bias tables
        "brz": inp("brz", [2 * N_GRU, N_TP]),
        "bxn": inp("bxn", [N_GRU, N_TP]),
        "bhhn": inp("bhhn", [N_GRU, 1]),
        "bz0": inp("bz0", [N_GRU, 1]),
    }
    out = nc.dram_tensor("out", [N_TP, OUT_DIM, B_LOC], BF16, kind="ExternalOutput").ap()

    with tile.TileContext(nc) as tc:
        _emit(nc, tc, d, out)
    n = _split_waits(nc, maxw=1)
    print(f"[kernel] split {n} excess sem-waits onto NOPs", flush=True)
    return nc


def _emit(nc, tc, d, out):
    with (
        tc.tile_pool(name="const", bufs=1) as cp,
        tc.tile_pool(name="work", bufs=2) as wp,
        tc.tile_pool(name="prz", bufs=3, space="PSUM") as przp,
        tc.tile_pool(name="phn", bufs=2, space="PSUM") as phnp,
        tc.tile_pool(name="pnx", bufs=2, space="PSUM") as pnxp,
        tc.tile_pool(name="ppr", bufs=1, space="PSUM") as pprp,
    ):
        def const_tile(name, shape, dt=F32):
            t = cp.tile(shape, dt, tag=name, name=name + "_c")
            nc.sync.dma_start(t[:], d[name][:])
            return t

        def const_rows(name, shape, r0, tag, dt=F32):
            t = cp.tile(shape, dt, tag=tag, name=tag + "_c")
            nc.sync.dma_start(t[:], d[name][r0 : r0 + shape[0], :])
            return t

        wz0 = const_tile("wz0t", [LATENT, N_GRU], BF16)
        w1 = [const_rows("w1t", [128, 4 * N_GRU], 128 * k, f"w1_{k}", BF16) for k in range(2)]
        w0 = [const_rows("w0t", [128, 2 * N_GRU], 128 * k, f"w0_{k}", BF16) for k in range(2)]
        wout = [const_rows("woutt", [128, OUT_DIM], 128 * k, f"wout_{k}", BF16) for k in range(2)]
        brz = [const_rows("brz", [128, N_TP], 128 * g, f"brz_{g}") for g in range(4)]
        bxn = [const_rows("bxn", [128, N_TP], 128 * c, f"bxn_{c}") for c in range(2)]
        bhhn = [const_rows("bhhn", [128, 1], 128 * c, f"bhhn_{c}") for c in range(2)]
        bz0 = [const_rows("bz0", [128, 1], 128 * c, f"bz0_{c}") for c in range(2)]

        z0sb = wp.tile([LATENT, B_LOC], BF16, tag="z0", bufs=1)
        nc.sync.dma_start(z0sb[:], d["z0t"][:])

        # ---- initial hidden: h[s][c] = tanh(Wz0 @ z0T + b_z0)  [128, SW]
        h = [[None, None] for _ in range(NS)]
        for s in range(NS):
            bs = slice(s * SW, (s + 1) * SW)
            for c in range(2):
                p = przp.tile([128, SW], F32, tag="prz", name="p0")
                nc.tensor.matmul(p[:], wz0[:, c * 128 : (c + 1) * 128], z0sb[:, bs],
                                 start=True, stop=True)
                hc = wp.tile([128, SW], BF16, tag=f"h{c}_{s}", name="h0")
                nc.scalar.activation(hc[:], p[:], AF.Tanh, bias=bz0[c][:, 0:1])
                h[s][c] = hc

        # Prime the nx psum banks' has_written bits with dummy matmuls so
        # later start=False matmuls accumulate onto DVE-written data.
        primed = []
        if not USE_TT_ADD:
            for i in range(2 * 2 * NS):  # cover all rotating bufs of the tag
                pp = pnxp.tile([128, SW], F32, tag="pnx", name="prime")
                nc.tensor.matmul(pp[:], w1[0][:, 0:128], z0sb[:, 0:SW],
                                 start=True, stop=True)
                primed.append(pp)

        sub_eng = nc.gpsimd if SUB_ENGINE == "gpsimd" else nc.vector

        for t in range(N_TP):
            first = t == 0
            # --- r,z gate psums + activations
            rt = [[None, None] for _ in range(NS)]
            zt = [[None, None] for _ in range(NS)]
            for s in range(NS):
                bs = slice(s * SW, (s + 1) * SW)
                for g in range(4):  # r0 r1 z0 z1
                    if first:
                        wk, col = w0, slice(g * 128, (g + 1) * 128)
                    else:
                        wk, col = w1, slice(g * 128, (g + 1) * 128)
                    p = przp.tile([128, SW], F32, tag="prz", name="prz")
                    nc.tensor.matmul(p[:], wk[0][:, col], h[s][0][:],
                                     start=True, stop=False)
                    nc.tensor.matmul(p[:], wk[1][:, col], h[s][1][:],
                                     start=False, stop=True)
                    gg = wp.tile([128, SW], BF16, tag=f"g{g}_{s}", name="gact")
                    nc.scalar.activation(gg[:], p[:], AF.Sigmoid,
                                         bias=brz[g][:, t : t + 1])
                    if g < 2:
                        rt[s][g] = gg
                    else:
                        zt[s][g - 2] = gg

            # --- hn psums
            phn = [[None, None] for _ in range(NS)]
            for s in range(NS):
                for c in range(2):
                    col = slice(512 + c * 128, 512 + (c + 1) * 128)
                    p = phnp.tile([128, SW], F32, tag="phn", name="phn")
                    nc.tensor.matmul(p[:], w1[0][:, col], h[s][0][:],
                                     start=True, stop=False)
                    nc.tensor.matmul(p[:], w1[1][:, col], h[s][1][:],
                                     start=False, stop=True)
                    phn[s][c] = p

            # --- n gate: t1 = (hn + b_hhn)*r ; t2 = t1 + xn ; n = tanh(t2+bxn)
            n_t = [[None, None] for _ in range(NS)]
            for s in range(NS):
                for c in range(2):
                    xcol = slice(768 + c * 128, 768 + (c + 1) * 128)
                    if first:
                        # no xn term at t=0 (cur = 0, no h-dependence)
                        t1 = wp.tile([128, SW], BF16, tag=f"t1_{c}_{s}", name="t1")
                        nc.vector.scalar_tensor_tensor(
                            t1[:], phn[s][c][:], bhhn[c][:, 0:1], rt[s][c][:],
                            ALU.add, ALU.mult)
                        src = t1
                    elif USE_TT_ADD:
                        t1 = wp.tile([128, SW], BF16, tag=f"t1_{c}_{s}", name="t1")
                        nc.vector.scalar_tensor_tensor(
                            t1[:], phn[s][c][:], bhhn[c][:, 0:1], rt[s][c][:],
                            ALU.add, ALU.mult)
                        px = pnxp.tile([128, SW], F32, tag="pnx", name="pnx")
                        nc.tensor.matmul(px[:], w1[0][:, xcol], h[s][0][:],
                                         start=True, stop=False)
                        nc.tensor.matmul(px[:], w1[1][:, xcol], h[s][1][:],
                                         start=False, stop=True)
                        t2 = wp.tile([128, SW], F32, tag=f"t2_{c}_{s}", name="t2")
                        nc.vector.tensor_tensor(t2[:], t1[:], px[:], ALU.add)
                        src = t2
                    else:
                        # STT writes t1 straight into the (primed) psum bank,
                        # then the xn matmuls accumulate onto it.
                        px = pnxp.tile([128, SW], F32, tag="pnx", name="pnx")
                        nc.vector.scalar_tensor_tensor(
                            px[:], phn[s][c][:], bhhn[c][:, 0:1], rt[s][c][:],
                            ALU.add, ALU.mult)
                        nc.tensor.matmul(px[:], w1[0][:, xcol], h[s][0][:],
                                         start=False, stop=False,
                                         skip_group_check=True)
                        nc.tensor.matmul(px[:], w1[1][:, xcol], h[s][1][:],
                                         start=False, stop=True,
                                         skip_group_check=True)
                        src = px
                    nt = wp.tile([128, SW], BF16, tag=f"n_{c}_{s}", name="nt")
                    nc.scalar.activation(nt[:], src[:], AF.Tanh,
                                         bias=bxn[c][:, t : t + 1])
                    n_t[s][c] = nt

            # --- blend h' = n + z*(h-n)
            h_new = [[None, None] for _ in range(NS)]
            for s in range(NS):
                for c in range(2):
                    dt_ = wp.tile([128, SW], BF16, tag=f"d_{c}_{s}", name="dt")
                    sub_eng.tensor_tensor(dt_[:], h[s][c][:], n_t[s][c][:], ALU.subtract)
                    e = wp.tile([128, SW], BF16, tag=f"e_{c}_{s}", name="et")
                    nc.vector.tensor_tensor(e[:], zt[s][c][:], dt_[:], ALU.mult)
                    hc = wp.tile([128, SW], BF16, tag=f"h{c}_{s}", name="hn2")
                    nc.vector.tensor_tensor(hc[:], e[:], n_t[s][c][:], ALU.add)
                    h_new[s][c] = hc
            h = h_new

            # --- pred = W_out @ h' (b_out added host-side) -> bf16 -> DRAM
            for s in range(NS):
                bs = slice(s * SW, (s + 1) * SW)
                p = pprp.tile([OUT_DIM, SW], F32, tag="ppr", name="ppr")
                nc.tensor.matmul(p[:], wout[0][:, :], h[s][0][:],
                                 start=True, stop=False)
                nc.tensor.matmul(p[:], wout[1][:, :], h[s][1][:],
                                 start=False, stop=True)
                pr = wp.tile([OUT_DIM, SW], BF16, tag=f"pred_{s}", name="pr")
                nc.vector.tensor_copy(pr[:], p[:])
                nc.sync.dma_start(out[t][:, bs], pr[:])


_CACHE = {}


def _prep_host(z0, tps_to_pred, W_z0, b_z0, W_ih, b_ih, W_hh, b_hh, W_out, b_out):
    import ml_dtypes

    f = np.float32
    bf = ml_dtypes.bfloat16
    z0 = np.asarray(z0, f)
    tps = np.asarray(tps_to_pred, f)
    W_z0, b_z0 = np.asarray(W_z0, f), np.asarray(b_z0, f)
    W_ih, b_ih = np.asarray(W_ih, f), np.asarray(b_ih, f)
    W_hh, b_hh = np.asarray(W_hh, f), np.asarray(b_hh, f)
    W_out, b_out = np.asarray(W_out, f), np.asarray(b_out, f)

    G2 = 2 * N_GRU
    Wihp = W_ih[:, :OUT_DIM]  # [768, 64]
    wt = W_ih[:, OUT_DIM]  # [768]
    Weff_rz = W_hh[:G2] + Wihp[:G2] @ W_out  # [512, 256]
    Wxnh = Wihp[G2:] @ W_out  # [256, 256]
    W1 = np.concatenate([Weff_rz, W_hh[G2:], Wxnh], axis=0)  # [1024, 256]
    w1t = np.ascontiguousarray(W1.T).astype(bf)  # [256, 1024]
    w0t = np.ascontiguousarray(W_hh[:G2].T).astype(bf)  # [256, 512]
    woutt = np.ascontiguousarray(W_out.T).astype(bf)  # [256, 64]
    wz0t = np.ascontiguousarray(W_z0.T).astype(bf)  # [128, 256]

    cb = Wihp @ b_out  # [768]
    bias_all = b_ih[:, None] + wt[:, None] * tps[None, :]  # [768, 200]
    brz = bias_all[:G2] + b_hh[:G2, None]
    brz[:, 1:] += cb[:G2, None]
    bxn = bias_all[G2:].copy()
    bxn[:, 1:] += cb[G2:, None]

    shared = {
        "wz0t": wz0t,
        "w1t": w1t,
        "w0t": w0t,
        "woutt": woutt,
        "brz": np.ascontiguousarray(brz, f),
        "bxn": np.ascontiguousarray(bxn, f),
        "bhhn": np.ascontiguousarray(b_hh[G2:].reshape(N_GRU, 1)),
        "bz0": np.ascontiguousarray(b_z0.reshape(N_GRU, 1)),
    }
    z0f = z0.reshape(B_FULL, LATENT)
    in_maps = []
    for i in range(N_CORES):
        m = dict(shared)
        m["z0t"] = np.ascontiguousarray(z0f[i * B_LOC : (i + 1) * B_LOC].T).astype(bf)
        in_maps.append(m)
    return in_maps, b_out


def _run(in_maps, **spmd_kwargs):
    if "nc" not in _CACHE:
        _CACHE["nc"] = _build_module()
    return run_bass_kernel_spmd(_CACHE["nc"], in_maps, list(range(N_CORES)), **spmd_kwargs)


def _gather(res, b_out):
    outp = np.empty((B_FULL, N_TP, OUT_DIM), np.float32)
    for i in range(N_CORES):
        o = np.asarray(res.results[i]["out"]).astype(np.float32)  # [200, 64, 1024]
        outp[i * B_LOC : (i + 1) * B_LOC] = o.transpose(2, 0, 1)
    outp += b_out[None, None, :]
    return outp.reshape(64, 128, N_TP, OUT_DIM)


def kernel(**inputs):
    in_maps, b_out = _prep_host(**inputs)
    res = _run(in_maps)
    return _gather(res, b_out)


def kernel_profiled(**inputs):
    """Like kernel(), but requests an NTFF trace; returns (output, results)."""
    in_maps, b_out = _prep_host(**inputs)
    res = _run(in_maps, trace=True)
    return _gather(res, b_out), res


# revision 7
# speedup vs baseline: 1.8849x; 1.5856x over previous
"""
Trainium2 Bass kernel for the Decoder_RNN_Simple problem (v2).

Math (per flat-batch element b, reference semantics):
  hidden0 = tanh(W_z0 @ z0 + b_z0)                       # [256]
  cur0 = 0
  for t in 0..199:
    x = [cur, tps[t]]                                    # [65]
    gx = W_ih @ x + b_ih ; gh = W_hh @ hidden + b_hh     # [768]
    r = sig(gx_r + gh_r); z = sig(gx_z + gh_z)
    n = tanh(gx_n + r * gh_n)
    h' = (1-z)*n + z*h ; pred = W_out @ h' + b_out       # [64]

Mapping (data-parallel over the flat batch of 8192 across 8 cores,
1024 rows per core, 2 independent 512-column streams per core;
[gates, batch] on-chip layout, bf16 everywhere except PSUM/biases):

  - cur_t = pred_{t-1} = W_out @ h_t + b_out for t>=1 is folded into
    ALL gates: r/z use W_eff = W_hh_rz + W_ih_rz@W_out; the n-gate x
    part uses W_xnh = W_ih_n@W_out so pred never feeds back on-chip.
    The n gate needs xn separate from hn: n = tanh(xn + r*(hn+b_hhn)).
  - t-dependent bias terms are host-precomputed [gate, 200] tables and
    applied through the ACT bias operand.
  - The n-gate sum (xn + t1) is formed in PSUM: the DVE STT writes
    t1 = (hn+b_hhn)*r into a PSUM bank whose has_written bits are
    pre-primed, then the xn matmuls accumulate onto it (start=False).
  - pred = W_out@h' + (b_out added on host) is DMA'd straight from
    PSUM to DRAM; it is not on the recurrence critical path.
"""

import sys

_TRN = "/opt/trn_rl_repo"
if _TRN not in sys.path:
    sys.path.insert(0, _TRN)

import numpy as np

import concourse.bass as bass
import concourse.mybir as mybir
import concourse.tile as tile
from concourse.vector_clock import ScopedClock
from concourse.bass_utils import run_bass_kernel_spmd

N_CORES = 8
LATENT = 128
OUT_DIM = 64
N_GRU = 256
N_TP = 200
B_FULL = 64 * 128
B_LOC = B_FULL // N_CORES  # 1024
SW = 512  # batch columns per stream
NS = 2  # streams
F32 = mybir.dt.float32
BF16 = mybir.dt.bfloat16
AF = mybir.ActivationFunctionType
ALU = mybir.AluOpType

# If the PE-accumulate-onto-DVE-written-PSUM trick fails on HW, set True
# to fall back to an explicit DVE add for t2 = t1 + xn.
USE_TT_ADD = False
# Engine for the blend's d = h - n: "gpsimd" or "dve". gpsimd shares its
# SBUF port with the DVE and inflates concurrent DVE ops ~2x — keep "dve".
SUB_ENGINE = "dve"


# walrus rejects sem waits carried on the kernel-tail Drain instruction
# ("Too many sync wait commands"); move them onto NOPs, one wait each.
def _patched_drain_and_barrier(self, tick_clock, wait_clock):
    carrier = self.nc.sync.nop()
    wait_clock.add_sem_waits(carrier.ins, ScopedClock({None: tick_clock.global_clock}))
    si = carrier.ins.sync_info
    waits = list(si.on_wait) if si is not None else []
    if len(waits) > 1:
        si.on_wait = waits[:1]
        rest = waits[1:]
        while rest:
            extra = self.nc.sync.nop()
            extra.ins.sync_info = mybir.SyncInfo(on_wait=rest[:1], on_update=[])
            rest = rest[1:]
    self.nc.sync.drain()
    self.nc.all_engine_barrier()
    popped = self.nc._tile_sem_poison_stack.pop()
    assert popped is self._sem_poison
    self.nc.clear_and_free_semaphores(list(self.sems.allocated().values()))
    self.nc.all_engine_barrier()


tile.TileContext._drain_and_barrier = _patched_drain_and_barrier


def _split_waits(nc, maxw=1):
    """This walrus rejects instructions carrying more than a couple of sem
    waits; move the excess onto same-engine NOPs inserted just before."""
    k = 0
    for f in nc.m.functions:
        for bb in f.blocks:
            insts = bb.instructions
            out = []
            changed = False
            for inst in insts:
                si = inst.sync_info
                waits = list(si.on_wait) if si is not None else []
                if len(waits) > maxw:
                    si.on_wait = waits[-maxw:]
                    excess = waits[:-maxw]
                    while excess:
                        chunk, excess = excess[:maxw], excess[maxw:]
                        nop = mybir.InstNoOp(name=f"waitsplit_{k}", ins=[], outs=[])
                        k += 1
                        nop.engine = inst.engine
                        nop.sync_info = mybir.SyncInfo(on_wait=chunk, on_update=[])
                        out.append(nop)
                    changed = True
                out.append(inst)
            if changed:
                bb.instructions = out
    return k


def _build_module():
    nc = bass.Bass("TRN2", target_bir_lowering=False, debug=False, num_devices=N_CORES)

    def inp(name, shape, dt=F32):
        return nc.dram_tensor(name, shape, dt, kind="ExternalInput").ap()

    d = {
        # bf16 operands
        "z0t": inp("z0t", [LATENT, B_LOC], BF16),
        "wz0t": inp("wz0t", [LATENT, N_GRU], BF16),
        # [256, 1024]: cols 0:256 r(eff) 256:512 z(eff) 512:768 hn 768:1024 xnh
        "w1t": inp("w1t", [N_GRU, 4 * N_GRU], BF16),
        # [256, 512]: raw W_hh r,z for step 0
        "w0t": inp("w0t", [N_GRU, 2 * N_GRU], BF16),
        "woutt": inp("woutt", [N_GRU, OUT_DIM], BF16),
        # f32 bias tables
        "brz": inp("brz", [2 * N_GRU, N_TP]),
        "bxn": inp("bxn", [N_GRU, N_TP]),
        "bhhn": inp("bhhn", [N_GRU, 1]),
        "bz0": inp("bz0", [N_GRU, 1]),
    }
    out = nc.dram_tensor("out", [N_TP, OUT_DIM, B_LOC], BF16, kind="ExternalOutput").ap()

    with tile.TileContext(nc) as tc:
        _emit(nc, tc, d, out)
    n = _split_waits(nc, maxw=1)
    print(f"[kernel] split {n} excess sem-waits onto NOPs", flush=True)
    return nc


def _emit(nc, tc, d, out):
    with (
        tc.tile_pool(name="const", bufs=1) as cp,
        tc.tile_pool(name="work", bufs=2) as wp,
        tc.tile_pool(name="prz", bufs=3, space="PSUM") as przp,
        tc.tile_pool(name="phn", bufs=2, space="PSUM") as phnp,
        tc.tile_pool(name="pnx", bufs=2, space="PSUM") as pnxp,
        tc.tile_pool(name="ppr", bufs=1, space="PSUM") as pprp,
    ):
        def const_tile(name, shape, dt=F32):
            t = cp.tile(shape, dt, tag=name, name=name + "_c")
            nc.sync.dma_start(t[:], d[name][:])
            return t

        def const_rows(name, shape, r0, tag, dt=F32):
            t = cp.tile(shape, dt, tag=tag, name=tag + "_c")
            nc.sync.dma_start(t[:], d[name][r0 : r0 + shape[0], :])
            return t

        wz0 = const_tile("wz0t", [LATENT, N_GRU], BF16)
        w1 = [const_rows("w1t", [128, 4 * N_GRU], 128 * k, f"w1_{k}", BF16) for k in range(2)]
        w0 = [const_rows("w0t", [128, 2 * N_GRU], 128 * k, f"w0_{k}", BF16) for k in range(2)]
        wout = [const_rows("woutt", [128, OUT_DIM], 128 * k, f"wout_{k}", BF16) for k in range(2)]
        brz = [const_rows("brz", [128, N_TP], 128 * g, f"brz_{g}") for g in range(4)]
        bxn = [const_rows("bxn", [128, N_TP], 128 * c, f"bxn_{c}") for c in range(2)]
        bhhn = [const_rows("bhhn", [128, 1], 128 * c, f"bhhn_{c}") for c in range(2)]
        bz0 = [const_rows("bz0", [128, 1], 128 * c, f"bz0_{c}") for c in range(2)]

        z0sb = wp.tile([LATENT, B_LOC], BF16, tag="z0", bufs=1)
        nc.sync.dma_start(z0sb[:], d["z0t"][:])

        # ---- initial hidden: h[s][c] = tanh(Wz0 @ z0T + b_z0)  [128, SW]
        h = [[None, None] for _ in range(NS)]
        for s in range(NS):
            bs = slice(s * SW, (s + 1) * SW)
            for c in range(2):
                p = przp.tile([128, SW], F32, tag="prz", name="p0")
                nc.tensor.matmul(p[:], wz0[:, c * 128 : (c + 1) * 128], z0sb[:, bs],
                                 start=True, stop=True)
                hc = wp.tile([128, SW], BF16, tag=f"h{c}_{s}", name="h0")
                nc.scalar.activation(hc[:], p[:], AF.Tanh, bias=bz0[c][:, 0:1])
                h[s][c] = hc

        # Prime the nx psum banks' has_written bits with dummy matmuls so
        # later start=False matmuls accumulate onto DVE-written data.
        primed = []
        if not USE_TT_ADD:
            for i in range(2 * 2 * NS):  # cover all rotating bufs of the tag
                pp = pnxp.tile([128, SW], F32, tag="pnx", name="prime")
                nc.tensor.matmul(pp[:], w1[0][:, 0:128], z0sb[:, 0:SW],
                                 start=True, stop=True)
                primed.append(pp)

        sub_eng = nc.gpsimd if SUB_ENGINE == "gpsimd" else nc.vector

        def emit_pred(t, s, hs):
            """pred(t) = W_out @ h'(t); b_out is added host-side."""
            bs = slice(s * SW, (s + 1) * SW)
            p = pprp.tile([OUT_DIM, SW], F32, tag="ppr", name="ppr")
            nc.tensor.matmul(p[:], wout[0][:, :], hs[0][:],
                             start=True, stop=False)
            nc.tensor.matmul(p[:], wout[1][:, :], hs[1][:],
                             start=False, stop=True)
            pr = wp.tile([OUT_DIM, SW], BF16, tag=f"pred_{s}", name="pr")
            nc.vector.tensor_copy(pr[:], p[:])
            nc.sync.dma_start(out[t][:, bs], pr[:])

        # Software-pipelined main loop: each (t, s) block is emitted as a
        # contiguous run on every engine so stream s1's matmuls overlap
        # stream s0's pointwise tail (and vice versa). pred for step t-1
        # is issued at the top of step t's block (same readiness point as
        # the step-t gate matmuls).
        for t in range(N_TP):
            first = t == 0
            h_new = [[None, None] for _ in range(NS)]
            for s in range(NS):
                if not first:
                    emit_pred(t - 1, s, h[s])

                # --- r,z gate psums + activations
                rt, zt = [None, None], [None, None]
                for g in range(4):  # r0 r1 z0 z1
                    wk = w0 if first else w1
                    col = slice(g * 128, (g + 1) * 128)
                    p = przp.tile([128, SW], F32, tag="prz", name="prz")
                    nc.tensor.matmul(p[:], wk[0][:, col], h[s][0][:],
                                     start=True, stop=False)
                    nc.tensor.matmul(p[:], wk[1][:, col], h[s][1][:],
                                     start=False, stop=True)
                    gg = wp.tile([128, SW], BF16, tag=f"g{g}_{s}", name="gact")
                    nc.scalar.activation(gg[:], p[:], AF.Sigmoid,
                                         bias=brz[g][:, t : t + 1])
                    if g < 2:
                        rt[g] = gg
                    else:
                        zt[g - 2] = gg

                # --- hn psums
                phn = [None, None]
                for c in range(2):
                    col = slice(512 + c * 128, 512 + (c + 1) * 128)
                    p = phnp.tile([128, SW], F32, tag="phn", name="phn")
                    nc.tensor.matmul(p[:], w1[0][:, col], h[s][0][:],
                                     start=True, stop=False)
                    nc.tensor.matmul(p[:], w1[1][:, col], h[s][1][:],
                                     start=False, stop=True)
                    phn[c] = p

                # --- n gate: t2 = (hn + b_hhn)*r + xn ; n = tanh(t2 + bxn)
                n_t = [None, None]
                for c in range(2):
                    xcol = slice(768 + c * 128, 768 + (c + 1) * 128)
                    if first:
                        # no xn term at t=0 (cur = 0, no h-dependence)
                        t1 = wp.tile([128, SW], BF16, tag=f"t1_{c}_{s}", name="t1")
                        nc.vector.scalar_tensor_tensor(
                            t1[:], phn[c][:], bhhn[c][:, 0:1], rt[c][:],
                            ALU.add, ALU.mult)
                        src = t1
                    elif USE_TT_ADD:
                        t1 = wp.tile([128, SW], BF16, tag=f"t1_{c}_{s}", name="t1")
                        nc.vector.scalar_tensor_tensor(
                            t1[:], phn[c][:], bhhn[c][:, 0:1], rt[c][:],
                            ALU.add, ALU.mult)
                        px = pnxp.tile([128, SW], F32, tag="pnx", name="pnx")
                        nc.tensor.matmul(px[:], w1[0][:, xcol], h[s][0][:],
                                         start=True, stop=False)
                        nc.tensor.matmul(px[:], w1[1][:, xcol], h[s][1][:],
                                         start=False, stop=True)
                        t2 = wp.tile([128, SW], F32, tag=f"t2_{c}_{s}", name="t2")
                        nc.vector.tensor_tensor(t2[:], t1[:], px[:], ALU.add)
                        src = t2
                    else:
                        # STT writes t1 straight into the (primed) psum bank,
                        # then the xn matmuls accumulate onto it.
                        px = pnxp.tile([128, SW], F32, tag="pnx", name="pnx")
                        nc.vector.scalar_tensor_tensor(
                            px[:], phn[c][:], bhhn[c][:, 0:1], rt[c][:],
                            ALU.add, ALU.mult)
                        nc.tensor.matmul(px[:], w1[0][:, xcol], h[s][0][:],
                                         start=False, stop=False,
                                         skip_group_check=True)
                        nc.tensor.matmul(px[:], w1[1][:, xcol], h[s][1][:],
                                         start=False, stop=True,
                                         skip_group_check=True)
                        src = px
                    nt = wp.tile([128, SW], BF16, tag=f"n_{c}_{s}", name="nt")
                    nc.scalar.activation(nt[:], src[:], AF.Tanh,
                                         bias=bxn[c][:, t : t + 1])
                    n_t[c] = nt

                # --- blend h' = n + z*(h-n)
                for c in range(2):
                    dt_ = wp.tile([128, SW], BF16, tag=f"d_{c}_{s}", name="dt")
                    sub_eng.tensor_tensor(dt_[:], h[s][c][:], n_t[c][:], ALU.subtract)
                    e = wp.tile([128, SW], BF16, tag=f"e_{c}_{s}", name="et")
                    nc.vector.tensor_tensor(e[:], zt[c][:], dt_[:], ALU.mult)
                    hc = wp.tile([128, SW], BF16, tag=f"h{c}_{s}", name="hn2")
                    nc.vector.tensor_tensor(hc[:], e[:], n_t[c][:], ALU.add)
                    h_new[s][c] = hc
            h = h_new

        for s in range(NS):
            emit_pred(N_TP - 1, s, h[s])


_CACHE = {}


def _prep_host(z0, tps_to_pred, W_z0, b_z0, W_ih, b_ih, W_hh, b_hh, W_out, b_out):
    import ml_dtypes

    f = np.float32
    bf = ml_dtypes.bfloat16
    z0 = np.asarray(z0, f)
    tps = np.asarray(tps_to_pred, f)
    W_z0, b_z0 = np.asarray(W_z0, f), np.asarray(b_z0, f)
    W_ih, b_ih = np.asarray(W_ih, f), np.asarray(b_ih, f)
    W_hh, b_hh = np.asarray(W_hh, f), np.asarray(b_hh, f)
    W_out, b_out = np.asarray(W_out, f), np.asarray(b_out, f)

    G2 = 2 * N_GRU
    Wihp = W_ih[:, :OUT_DIM]  # [768, 64]
    wt = W_ih[:, OUT_DIM]  # [768]
    Weff_rz = W_hh[:G2] + Wihp[:G2] @ W_out  # [512, 256]
    Wxnh = Wihp[G2:] @ W_out  # [256, 256]
    W1 = np.concatenate([Weff_rz, W_hh[G2:], Wxnh], axis=0)  # [1024, 256]
    w1t = np.ascontiguousarray(W1.T).astype(bf)  # [256, 1024]
    w0t = np.ascontiguousarray(W_hh[:G2].T).astype(bf)  # [256, 512]
    woutt = np.ascontiguousarray(W_out.T).astype(bf)  # [256, 64]
    wz0t = np.ascontiguousarray(W_z0.T).astype(bf)  # [128, 256]

    cb = Wihp @ b_out  # [768]
    bias_all = b_ih[:, None] + wt[:, None] * tps[None, :]  # [768, 200]
    brz = bias_all[:G2] + b_hh[:G2, None]
    brz[:, 1:] += cb[:G2, None]
    bxn = bias_all[G2:].copy()
    bxn[:, 1:] += cb[G2:, None]

    shared = {
        "wz0t": wz0t,
        "w1t": w1t,
        "w0t": w0t,
        "woutt": woutt,
        "brz": np.ascontiguousarray(brz, f),
        "bxn": np.ascontiguousarray(bxn, f),
        "bhhn": np.ascontiguousarray(b_hh[G2:].reshape(N_GRU, 1)),
        "bz0": np.ascontiguousarray(b_z0.reshape(N_GRU, 1)),
    }
    z0f = z0.reshape(B_FULL, LATENT)
    in_maps = []
    for i in range(N_CORES):
        m = dict(shared)
        m["z0t"] = np.ascontiguousarray(z0f[i * B_LOC : (i + 1) * B_LOC].T).astype(bf)
        in_maps.append(m)
    return in_maps, b_out


def _run(in_maps, **spmd_kwargs):
    if "nc" not in _CACHE:
        _CACHE["nc"] = _build_module()
    return run_bass_kernel_spmd(_CACHE["nc"], in_maps, list(range(N_CORES)), **spmd_kwargs)


def _gather(res, b_out):
    outp = np.empty((B_FULL, N_TP, OUT_DIM), np.float32)
    for i in range(N_CORES):
        o = np.asarray(res.results[i]["out"]).astype(np.float32)  # [200, 64, 1024]
        outp[i * B_LOC : (i + 1) * B_LOC] = o.transpose(2, 0, 1)
    outp += b_out[None, None, :]
    return outp.reshape(64, 128, N_TP, OUT_DIM)


def kernel(**inputs):
    in_maps, b_out = _prep_host(**inputs)
    res = _run(in_maps)
    return _gather(res, b_out)


def kernel_profiled(**inputs):
    """Like kernel(), but requests an NTFF trace; returns (output, results)."""
    in_maps, b_out = _prep_host(**inputs)
    res = _run(in_maps, trace=True)
    return _gather(res, b_out), res


# revision 14
# speedup vs baseline: 1.9376x; 1.0280x over previous
"""
Trainium2 Bass kernel for the Decoder_RNN_Simple problem (v2).

Math (per flat-batch element b, reference semantics):
  hidden0 = tanh(W_z0 @ z0 + b_z0)                       # [256]
  cur0 = 0
  for t in 0..199:
    x = [cur, tps[t]]                                    # [65]
    gx = W_ih @ x + b_ih ; gh = W_hh @ hidden + b_hh     # [768]
    r = sig(gx_r + gh_r); z = sig(gx_z + gh_z)
    n = tanh(gx_n + r * gh_n)
    h' = (1-z)*n + z*h ; pred = W_out @ h' + b_out       # [64]

Mapping (data-parallel over the flat batch of 8192 across 8 cores,
1024 rows per core, 2 independent 512-column streams per core;
[gates, batch] on-chip layout, bf16 everywhere except PSUM/biases):

  - cur_t = pred_{t-1} = W_out @ h_t + b_out for t>=1 is folded into
    ALL gates: r/z use W_eff = W_hh_rz + W_ih_rz@W_out; the n-gate x
    part uses W_xnh = W_ih_n@W_out so pred never feeds back on-chip.
    The n gate needs xn separate from hn: n = tanh(xn + r*(hn+b_hhn)).
  - t-dependent bias terms are host-precomputed [gate, 200] tables and
    applied through the ACT bias operand.
  - The n-gate sum (xn + t1) is formed in PSUM: the DVE STT writes
    t1 = (hn+b_hhn)*r into a PSUM bank whose has_written bits are
    pre-primed, then the xn matmuls accumulate onto it (start=False).
  - pred = W_out@h' + (b_out added on host) is DMA'd straight from
    PSUM to DRAM; it is not on the recurrence critical path.
"""

import sys

_TRN = "/opt/trn_rl_repo"
if _TRN not in sys.path:
    sys.path.insert(0, _TRN)

import numpy as np

import concourse.bass as bass
import concourse.mybir as mybir
import concourse.tile as tile
from concourse.vector_clock import ScopedClock
from concourse.bass_utils import run_bass_kernel_spmd

N_CORES = 8
LATENT = 128
OUT_DIM = 64
N_GRU = 256
N_TP = 200
B_FULL = 64 * 128
B_LOC = B_FULL // N_CORES  # 1024
SW = 512  # batch columns per stream
NS = 2  # streams
F32 = mybir.dt.float32
BF16 = mybir.dt.bfloat16
AF = mybir.ActivationFunctionType
ALU = mybir.AluOpType

# If the PE-accumulate-onto-DVE-written-PSUM trick fails on HW, set True
# to fall back to an explicit DVE add for t2 = t1 + xn.
USE_TT_ADD = False
# Engine for the blend's d = h - n: "gpsimd" or "dve". gpsimd shares its
# SBUF port with the DVE and inflates concurrent DVE ops ~2x — keep "dve".
SUB_ENGINE = "dve"


# walrus rejects sem waits carried on the kernel-tail Drain instruction
# ("Too many sync wait commands"); move them onto NOPs, one wait each.
def _patched_drain_and_barrier(self, tick_clock, wait_clock):
    carrier = self.nc.sync.nop()
    wait_clock.add_sem_waits(carrier.ins, ScopedClock({None: tick_clock.global_clock}))
    si = carrier.ins.sync_info
    waits = list(si.on_wait) if si is not None else []
    if len(waits) > 1:
        si.on_wait = waits[:1]
        rest = waits[1:]
        while rest:
            extra = self.nc.sync.nop()
            extra.ins.sync_info = mybir.SyncInfo(on_wait=rest[:1], on_update=[])
            rest = rest[1:]
    self.nc.sync.drain()
    self.nc.all_engine_barrier()
    popped = self.nc._tile_sem_poison_stack.pop()
    assert popped is self._sem_poison
    self.nc.clear_and_free_semaphores(list(self.sems.allocated().values()))
    self.nc.all_engine_barrier()


tile.TileContext._drain_and_barrier = _patched_drain_and_barrier


def _split_waits(nc, maxw=1):
    """This walrus rejects instructions carrying more than a couple of sem
    waits; move the excess onto same-engine NOPs inserted just before."""
    k = 0
    for f in nc.m.functions:
        for bb in f.blocks:
            insts = bb.instructions
            out = []
            changed = False
            for inst in insts:
                si = inst.sync_info
                waits = list(si.on_wait) if si is not None else []
                if len(waits) > maxw:
                    si.on_wait = waits[-maxw:]
                    excess = waits[:-maxw]
                    while excess:
                        chunk, excess = excess[:maxw], excess[maxw:]
                        nop = mybir.InstNoOp(name=f"waitsplit_{k}", ins=[], outs=[])
                        k += 1
                        nop.engine = inst.engine
                        nop.sync_info = mybir.SyncInfo(on_wait=chunk, on_update=[])
                        out.append(nop)
                    changed = True
                out.append(inst)
            if changed:
                bb.instructions = out
    return k


def _build_module():
    nc = bass.Bass("TRN2", target_bir_lowering=False, debug=False, num_devices=N_CORES)

    def inp(name, shape, dt=F32):
        return nc.dram_tensor(name, shape, dt, kind="ExternalInput").ap()

    d = {
        # bf16 operands
        "z0t": inp("z0t", [LATENT, B_LOC], BF16),
        "wz0t": inp("wz0t", [LATENT, N_GRU], BF16),
        # [256, 768]: cols 0:256 r(eff) 256:512 z(eff) 512:768 hn
        "w1t": inp("w1t", [N_GRU, 3 * N_GRU], BF16),
        # [64, 256]: xn weights vs pred (pred-feedback form)
        "wxnt": inp("wxnt", [OUT_DIM, N_GRU], BF16),
        # [256, 512]: raw W_hh r,z for step 0
        "w0t": inp("w0t", [N_GRU, 2 * N_GRU], BF16),
        "woutt": inp("woutt", [N_GRU, OUT_DIM], BF16),
        # f32 bias tables
        "brz": inp("brz", [2 * N_GRU, N_TP]),
        "bxn": inp("bxn", [N_GRU, N_TP]),
        "bhhn": inp("bhhn", [N_GRU, 1]),
        "bz0": inp("bz0", [N_GRU, 1]),
    }
    out = nc.dram_tensor("out", [N_TP, OUT_DIM, B_LOC], BF16, kind="ExternalOutput").ap()

    with tile.TileContext(nc) as tc:
        _emit(nc, tc, d, out)
    n = _split_waits(nc, maxw=1)
    print(f"[kernel] split {n} excess sem-waits onto NOPs", flush=True)
    return nc


def _emit(nc, tc, d, out):
    with (
        tc.tile_pool(name="const", bufs=1) as cp,
        tc.tile_pool(name="work", bufs=2) as wp,
        tc.tile_pool(name="prz", bufs=3, space="PSUM") as przp,
        tc.tile_pool(name="phn", bufs=2, space="PSUM") as phnp,
        tc.tile_pool(name="pnx", bufs=2, space="PSUM") as pnxp,
        tc.tile_pool(name="ppr", bufs=1, space="PSUM") as pprp,
    ):
        def const_tile(name, shape, dt=F32):
            t = cp.tile(shape, dt, tag=name, name=name + "_c")
            nc.sync.dma_start(t[:], d[name][:])
            return t

        def const_rows(name, shape, r0, tag, dt=F32):
            t = cp.tile(shape, dt, tag=tag, name=tag + "_c")
            nc.sync.dma_start(t[:], d[name][r0 : r0 + shape[0], :])
            return t

        wz0 = const_tile("wz0t", [LATENT, N_GRU], BF16)
        wxn = const_tile("wxnt", [OUT_DIM, N_GRU], BF16)
        w1 = [const_rows("w1t", [128, 3 * N_GRU], 128 * k, f"w1_{k}", BF16) for k in range(2)]
        w0 = [const_rows("w0t", [128, 2 * N_GRU], 128 * k, f"w0_{k}", BF16) for k in range(2)]
        wout = [const_rows("woutt", [128, OUT_DIM], 128 * k, f"wout_{k}", BF16) for k in range(2)]
        brz = [const_rows("brz", [128, N_TP], 128 * g, f"brz_{g}") for g in range(4)]
        bxn = [const_rows("bxn", [128, N_TP], 128 * c, f"bxn_{c}") for c in range(2)]
        bhhn = [const_rows("bhhn", [128, 1], 128 * c, f"bhhn_{c}") for c in range(2)]
        bz0 = [const_rows("bz0", [128, 1], 128 * c, f"bz0_{c}") for c in range(2)]

        z0sb = wp.tile([LATENT, B_LOC], BF16, tag="z0", bufs=1)
        nc.sync.dma_start(z0sb[:], d["z0t"][:])

        # ---- initial hidden: h[s][c] = tanh(Wz0 @ z0T + b_z0)  [128, SW]
        h = [[None, None] for _ in range(NS)]
        for s in range(NS):
            bs = slice(s * SW, (s + 1) * SW)
            for c in range(2):
                p = przp.tile([128, SW], F32, tag="prz", name="p0")
                nc.tensor.matmul(p[:], wz0[:, c * 128 : (c + 1) * 128], z0sb[:, bs],
                                 start=True, stop=True)
                hc = wp.tile([128, SW], BF16, tag=f"h{c}_{s}", name="h0")
                nc.scalar.activation(hc[:], p[:], AF.Tanh, bias=bz0[c][:, 0:1])
                h[s][c] = hc

        # Prime the nx psum banks' has_written bits with dummy matmuls so
        # later start=False matmuls accumulate onto DVE-written data.
        primed = []
        if not USE_TT_ADD:
            for i in range(2 * 2 * NS):  # cover all rotating bufs of the tag
                pp = pnxp.tile([128, SW], F32, tag="pnx", name="prime")
                nc.tensor.matmul(pp[:], w1[0][:, 0:128], z0sb[:, 0:SW],
                                 start=True, stop=True)
                primed.append(pp)

        sub_eng = nc.gpsimd if SUB_ENGINE == "gpsimd" else nc.vector

        def emit_pred(t, s, hs):
            """pred(t) = W_out @ h'(t); b_out is added host-side."""
            bs = slice(s * SW, (s + 1) * SW)
            p = pprp.tile([OUT_DIM, SW], F32, tag="ppr", name="ppr")
            nc.tensor.matmul(p[:], wout[0][:, :], hs[0][:],
                             start=True, stop=False)
            nc.tensor.matmul(p[:], wout[1][:, :], hs[1][:],
                             start=False, stop=True)
            pr = wp.tile([OUT_DIM, SW], BF16, tag=f"pred_{s}", name="pr")
            if s == 0:
                nc.vector.tensor_copy(pr[:], p[:])
            else:
                nc.scalar.copy(pr[:], p[:])
            nc.sync.dma_start(out[t][:, bs], pr[:])
            return pr

        # Software-pipelined main loop: each (t, s) block is emitted as a
        # contiguous run on every engine so stream s1's matmuls overlap
        # stream s0's pointwise tail (and vice versa). pred for step t-1
        # is issued at the top of step t's block (same readiness point as
        # the step-t gate matmuls).
        for t in range(N_TP):
            first = t == 0
            h_new = [[None, None] for _ in range(NS)]
            for s in range(NS):
                pr = None
                if not first:
                    pr = emit_pred(t - 1, s, h[s])

                # --- r,z gate psums + activations
                rt, zt = [None, None], [None, None]
                for g in range(4):  # r0 r1 z0 z1
                    wk = w0 if first else w1
                    col = slice(g * 128, (g + 1) * 128)
                    p = przp.tile([128, SW], F32, tag="prz", name="prz")
                    nc.tensor.matmul(p[:], wk[0][:, col], h[s][0][:],
                                     start=True, stop=False)
                    nc.tensor.matmul(p[:], wk[1][:, col], h[s][1][:],
                                     start=False, stop=True)
                    gg = wp.tile([128, SW], BF16, tag=f"g{g}_{s}", name="gact")
                    nc.scalar.activation(gg[:], p[:], AF.Sigmoid,
                                         bias=brz[g][:, t : t + 1])
                    if g < 2:
                        rt[g] = gg
                    else:
                        zt[g - 2] = gg

                # --- hn psums
                phn = [None, None]
                for c in range(2):
                    col = slice(512 + c * 128, 512 + (c + 1) * 128)
                    p = phnp.tile([128, SW], F32, tag="phn", name="phn")
                    nc.tensor.matmul(p[:], w1[0][:, col], h[s][0][:],
                                     start=True, stop=False)
                    nc.tensor.matmul(p[:], w1[1][:, col], h[s][1][:],
                                     start=False, stop=True)
                    phn[c] = p

                # --- n gate: t2 = (hn + b_hhn)*r + xn ; n = tanh(t2 + bxn)
                # xn = W_xn @ pred(t-1) (K=64, pred-feedback form)
                n_t = [None, None]
                for c in range(2):
                    xcol = slice(c * 128, (c + 1) * 128)
                    if first:
                        # no xn term at t=0 (cur = 0)
                        t1 = wp.tile([128, SW], BF16, tag=f"t1_{c}_{s}", name="t1")
                        nc.vector.scalar_tensor_tensor(
                            t1[:], phn[c][:], bhhn[c][:, 0:1], rt[c][:],
                            ALU.add, ALU.mult)
                        src = t1
                    elif USE_TT_ADD:
                        t1 = wp.tile([128, SW], BF16, tag=f"t1_{c}_{s}", name="t1")
                        nc.vector.scalar_tensor_tensor(
                            t1[:], phn[c][:], bhhn[c][:, 0:1], rt[c][:],
                            ALU.add, ALU.mult)
                        px = pnxp.tile([128, SW], F32, tag="pnx", name="pnx")
                        nc.tensor.matmul(px[:], wxn[:, xcol], pr[:],
                                         start=True, stop=True)
                        t2 = wp.tile([128, SW], F32, tag=f"t2_{c}_{s}", name="t2")
                        nc.vector.tensor_tensor(t2[:], t1[:], px[:], ALU.add)
                        src = t2
                    else:
                        # STT writes t1 straight into the (primed) psum bank,
                        # then the xn matmul accumulates onto it.
                        px = pnxp.tile([128, SW], F32, tag="pnx", name="pnx")
                        nc.vector.scalar_tensor_tensor(
                            px[:], phn[c][:], bhhn[c][:, 0:1], rt[c][:],
                            ALU.add, ALU.mult)
                        nc.tensor.matmul(px[:], wxn[:, xcol], pr[:],
                                         start=False, stop=True,
                                         skip_group_check=True)
                        src = px
                    nt = wp.tile([128, SW], BF16, tag=f"n_{c}_{s}", name="nt")
                    nc.scalar.activation(nt[:], src[:], AF.Tanh,
                                         bias=bxn[c][:, t : t + 1])
                    n_t[c] = nt

                # --- blend h' = n + z*(h-n)
                for c in range(2):
                    dt_ = wp.tile([128, SW], BF16, tag=f"d_{c}_{s}", name="dt")
                    sub_eng.tensor_tensor(dt_[:], h[s][c][:], n_t[c][:], ALU.subtract)
                    e = wp.tile([128, SW], BF16, tag=f"e_{c}_{s}", name="et")
                    nc.vector.tensor_tensor(e[:], zt[c][:], dt_[:], ALU.mult)
                    hc = wp.tile([128, SW], BF16, tag=f"h{c}_{s}", name="hn2")
                    nc.vector.tensor_tensor(hc[:], e[:], n_t[c][:], ALU.add)
                    h_new[s][c] = hc
            h = h_new

        for s in range(NS):
            emit_pred(N_TP - 1, s, h[s])


_CACHE = {}


def _prep_host(z0, tps_to_pred, W_z0, b_z0, W_ih, b_ih, W_hh, b_hh, W_out, b_out):
    import ml_dtypes

    f = np.float32
    bf = ml_dtypes.bfloat16
    z0 = np.asarray(z0, f)
    tps = np.asarray(tps_to_pred, f)
    W_z0, b_z0 = np.asarray(W_z0, f), np.asarray(b_z0, f)
    W_ih, b_ih = np.asarray(W_ih, f), np.asarray(b_ih, f)
    W_hh, b_hh = np.asarray(W_hh, f), np.asarray(b_hh, f)
    W_out, b_out = np.asarray(W_out, f), np.asarray(b_out, f)

    G2 = 2 * N_GRU
    Wihp = W_ih[:, :OUT_DIM]  # [768, 64]
    wt = W_ih[:, OUT_DIM]  # [768]
    Weff_rz = W_hh[:G2] + Wihp[:G2] @ W_out  # [512, 256]
    W1 = np.concatenate([Weff_rz, W_hh[G2:]], axis=0)  # [768, 256]
    w1t = np.ascontiguousarray(W1.T).astype(bf)  # [256, 768]
    wxnt = np.ascontiguousarray(Wihp[G2:].T).astype(bf)  # [64, 256]
    w0t = np.ascontiguousarray(W_hh[:G2].T).astype(bf)  # [256, 512]
    woutt = np.ascontiguousarray(W_out.T).astype(bf)  # [256, 64]
    wz0t = np.ascontiguousarray(W_z0.T).astype(bf)  # [128, 256]

    cb = Wihp @ b_out  # [768]
    bias_all = b_ih[:, None] + wt[:, None] * tps[None, :]  # [768, 200]
    brz = bias_all[:G2] + b_hh[:G2, None]
    brz[:, 1:] += cb[:G2, None]
    bxn = bias_all[G2:].copy()
    bxn[:, 1:] += cb[G2:, None]

    shared = {
        "wz0t": wz0t,
        "w1t": w1t,
        "wxnt": wxnt,
        "w0t": w0t,
        "woutt": woutt,
        "brz": np.ascontiguousarray(brz, f),
        "bxn": np.ascontiguousarray(bxn, f),
        "bhhn": np.ascontiguousarray(b_hh[G2:].reshape(N_GRU, 1)),
        "bz0": np.ascontiguousarray(b_z0.reshape(N_GRU, 1)),
    }
    z0f = z0.reshape(B_FULL, LATENT)
    in_maps = []
    for i in range(N_CORES):
        m = dict(shared)
        m["z0t"] = np.ascontiguousarray(z0f[i * B_LOC : (i + 1) * B_LOC].T).astype(bf)
        in_maps.append(m)
    return in_maps, b_out


def _run(in_maps, **spmd_kwargs):
    if "nc" not in _CACHE:
        _CACHE["nc"] = _build_module()
    return run_bass_kernel_spmd(_CACHE["nc"], in_maps, list(range(N_CORES)), **spmd_kwargs)


def _gather(res, b_out):
    outp = np.empty((B_FULL, N_TP, OUT_DIM), np.float32)
    for i in range(N_CORES):
        o = np.asarray(res.results[i]["out"]).astype(np.float32)  # [200, 64, 1024]
        outp[i * B_LOC : (i + 1) * B_LOC] = o.transpose(2, 0, 1)
    outp += b_out[None, None, :]
    return outp.reshape(64, 128, N_TP, OUT_DIM)


def kernel(**inputs):
    in_maps, b_out = _prep_host(**inputs)
    res = _run(in_maps)
    return _gather(res, b_out)


def kernel_profiled(**inputs):
    """Like kernel(), but requests an NTFF trace; returns (output, results)."""
    in_maps, b_out = _prep_host(**inputs)
    res = _run(in_maps, trace=True)
    return _gather(res, b_out), res
